# revision 4
# baseline (speedup 1.0000x reference)
"""Trainium2 Bass kernel for nn_LinearNet (complex double-linear).

Reference math (N = 4096):
    R_r = x @ W_r^T          R_i = x @ W_i^T
    C_r = W_r^T @ R_r - W_i^T @ R_i
    C_i = W_r^T @ R_i + W_i^T @ R_r
    out = concat([C_r, C_i], axis=1)                    # [N, 2N]

Sharding: core c owns output columns S_c = [c*512, (c+1)*512) of both C_r
and C_i.  No inter-core communication.

Pass 2 uses the 3-multiplication complex trick (Karatsuba):
    t1 = W_r^T R_r,  t2 = W_i^T R_i,  t3 = (W_r+W_i)^T (R_r+R_i)
    C_r = t1 - t2,   C_i = t3 - t1 - t2
which cuts the total matmul volume from 6 to 5 N^3-units.  All matmul
operands are bf16 (full PE speed, half the DMA/SBUF of f32); PSUM
accumulation stays fp32.  R never leaves SBUF: pass 1 evacuates PSUM
straight into the bf16 SBUF tiles pass 2 consumes, and R_s = R_r + R_i
is formed on the Pool engine during pass 1.

Host-side layout prep turns every device DMA into a contiguous slab.
Startup DMAs are fine-grained so the first matmul issues right after the
~8 us runtime boot; pass 2's first weight chunk is prefetched mid-pass-1
(issued on the Pool engine after ms=0 so it doesn't compete with the
startup-critical loads).
"""

import numpy as np

N = 4096
P = 128
NCORES = 8
SH = N // NCORES  # 512 output columns per core
KT = N // P  # 32 contraction tiles
MSUP = 256  # pass-1 output-row super tile (2 PSUM pairs live)
KH = KT // 2  # k-tiles per x half-chunk

_CACHE = {}


def _build():
    import concourse.mybir as mybir
    import concourse.tile as tile
    from concourse import bacc

    f32 = mybir.dt.float32
    bf16 = mybir.dt.bfloat16
    sub = mybir.AluOpType.subtract

    nc = bacc.Bacc()
    # xP[ms*128+p, k*256+c] = x[ms*256+c, k*128+p]
    xP = nc.declare_dram_parameter("xP", [N // MSUP * P, KT * MSUP], bf16,
                                   isOutput=False)
    # wrT[p, k*512+n] = W_r[c*512+n, k*128+p]   (per-core slice)
    wrT = nc.declare_dram_parameter("wrT", [P, KT * SH], bf16, isOutput=False)
    wiT = nc.declare_dram_parameter("wiT", [P, KT * SH], bf16, isOutput=False)
    # wrP[a*128+p, j*128+c] = W_r[j*128+p, a*128+c]   (tile-transposed full W)
    wrP = nc.declare_dram_parameter("wrP", [N, N], bf16, isOutput=False)
    wiP = nc.declare_dram_parameter("wiP", [N, N], bf16, isOutput=False)
    wsP = nc.declare_dram_parameter("wsP", [N, N], bf16, isOutput=False)
    out_r = nc.declare_dram_parameter("out_r", [N, SH], f32, isOutput=True)
    out_i = nc.declare_dram_parameter("out_i", [N, SH], f32, isOutput=True)

    with tile.TileContext(nc) as tc:
        # R lives in SBUF for the whole kernel: 3 x 32 KB/partition (bf16).
        # ws0: pass-2 a=0 weight chunk, prefetched during pass 1.
        with (
            tc.tile_pool(name="rres", bufs=1) as r_pool,
            tc.tile_pool(name="ws0", bufs=1) as ws0_pool,
        ):
            rr_sb = r_pool.tile([P, KT * SH], bf16)
            ri_sb = r_pool.tile([P, KT * SH], bf16)
            rs_sb = r_pool.tile([P, KT * SH], bf16)
            wrc0 = ws0_pool.tile([P, N], bf16)
            wic0 = ws0_pool.tile([P, N], bf16)
            wsc0 = ws0_pool.tile([P, N], bf16)

            # ---------- pass 1: R[:, S_c] = x @ W[S_c, :]^T ----------
            # psum[m, n] = sum_kk x[ms*256+m, k*128+kk] * W[n, k*128+kk]
            with (
                tc.tile_pool(name="wt", bufs=1) as wt_pool,
                tc.tile_pool(name="xs", bufs=2) as xs_pool,
                tc.tile_pool(name="ps1", bufs=4, space="PSUM") as ps1,
            ):
                wrT_sb = wt_pool.tile([P, KT * SH], bf16)
                wiT_sb = wt_pool.tile([P, KT * SH], bf16)
                # fine-grained k-sliced loads, wr/wi alternating, so the
                # first matmuls can start as soon as the k=0..3 slices land
                for k in range(0, KT, 4):
                    s = slice(k * SH, (k + 4) * SH)
                    nc.scalar.dma_start(wrT_sb[:, s], wrT[:, s])
                    nc.scalar.dma_start(wiT_sb[:, s], wiT[:, s])

                msub = MSUP // P  # 2
                for ms in range(N // MSUP):  # 16
                    xc = [
                        xs_pool.tile([P, KH * MSUP], bf16, tag="xc",
                                     name=f"xc{_h}")
                        for _h in range(2)
                    ]
                    for kh in range(2):
                        for q in range(2):
                            s = slice(q * KH * MSUP // 2,
                                      (q + 1) * KH * MSUP // 2)
                            ds = slice(kh * KH * MSUP + q * KH * MSUP // 2,
                                       kh * KH * MSUP + (q + 1) * KH * MSUP // 2)
                            nc.sync.dma_start(
                                xc[kh][:, s], xP[ms * P : (ms + 1) * P, ds]
                            )
                    acc_r = [
                        ps1.tile([P, SH], f32, tag="ps_r", name=f"accr{_s}")
                        for _s in range(msub)
                    ]
                    acc_i = [
                        ps1.tile([P, SH], f32, tag="ps_i", name=f"acci{_s}")
                        for _s in range(msub)
                    ]
                    for k in range(KT):
                        kh, kk = divmod(k, KH)
                        first, last = k == 0, k == KT - 1
                        for s in range(msub):
                            lhs = xc[kh][
                                :, kk * MSUP + s * P : kk * MSUP + (s + 1) * P
                            ]
                            nc.tensor.matmul(
                                acc_r[s][:],
                                lhs,
                                wrT_sb[:, k * SH : (k + 1) * SH],
                                start=first,
                                stop=last,
                            )
                            nc.tensor.matmul(
                                acc_i[s][:],
                                lhs,
                                wiT_sb[:, k * SH : (k + 1) * SH],
                                start=first,
                                stop=last,
                            )
                    for s in range(msub):
                        mt = ms * msub + s
                        sl = slice(mt * SH, (mt + 1) * SH)
                        nc.scalar.copy(rr_sb[:, sl], acc_r[s][:])
                        nc.vector.tensor_copy(ri_sb[:, sl], acc_i[s][:])
                        nc.gpsimd.tensor_add(
                            rs_sb[:, sl], rr_sb[:, sl], ri_sb[:, sl]
                        )
                    if ms == 0:
                        # prefetch pass-2 a=0 weights; issued on the Pool
                        # engine AFTER ms=0's rs adds (program order) so the
                        # startup-critical loads above get the HBM first
                        nc.gpsimd.dma_start(wrc0[:], wrP[0:P, :])
                        nc.gpsimd.dma_start(wic0[:], wiP[0:P, :])
                        nc.gpsimd.dma_start(wsc0[:], wsP[0:P, :])

            # ---------- pass 2: C[:, S_c] = W^T @ R (Karatsuba) ----------
            # t1[a,b] = sum_j wr[j,a] rr[j,b]; t2: wi,ri; t3: ws,rs
            with (
                tc.tile_pool(name="ws", bufs=2) as ws_pool,
                tc.tile_pool(name="ev2", bufs=3) as ev2_pool,
                tc.tile_pool(name="ps2", bufs=2, space="PSUM") as ps2,
            ):
                for a in range(N // P):  # 32
                    asl = slice(a * P, (a + 1) * P)
                    if a == 0:
                        wrc, wic, wsc = wrc0, wic0, wsc0
                    else:
                        wrc = ws_pool.tile([P, N], bf16, tag="wrc")
                        wic = ws_pool.tile([P, N], bf16, tag="wic")
                        wsc = ws_pool.tile([P, N], bf16, tag="wsc")
                        nc.sync.dma_start(wrc[:], wrP[asl, :])
                        nc.scalar.dma_start(wic[:], wiP[asl, :])
                        nc.sync.dma_start(wsc[:], wsP[asl, :])
                    t1 = ps2.tile([P, SH], f32, tag="t1")
                    t2 = ps2.tile([P, SH], f32, tag="t2")
                    t3 = ps2.tile([P, SH], f32, tag="t3")
                    for j in range(KT):
                        first, last = j == 0, j == KT - 1
                        wsl = slice(j * P, (j + 1) * P)
                        rsl = slice(j * SH, (j + 1) * SH)
                        nc.tensor.matmul(
                            t1[:], wrc[:, wsl], rr_sb[:, rsl],
                            start=first, stop=last,
                        )
                        nc.tensor.matmul(
                            t2[:], wic[:, wsl], ri_sb[:, rsl],
                            start=first, stop=last,
                        )
                        nc.tensor.matmul(
                            t3[:], wsc[:, wsl], rs_sb[:, rsl],
                            start=first, stop=last,
                        )
                    s1 = ev2_pool.tile([P, SH], f32, tag="s1")
                    cr = ev2_pool.tile([P, SH], f32, tag="cr")
                    ci = ev2_pool.tile([P, SH], f32, tag="ci")
                    nc.scalar.copy(s1[:], t1[:])
                    nc.vector.tensor_tensor(cr[:], s1[:], t2[:], sub)
                    nc.vector.tensor_tensor(ci[:], t3[:], s1[:], sub)
                    nc.vector.tensor_tensor(ci[:], ci[:], t2[:], sub)
                    nc.scalar.dma_start(out_r[asl, :], cr[:])
                    nc.scalar.dma_start(out_i[asl, :], ci[:])

    nc.finalize()
    return nc


def _get_nc():
    if "nc" not in _CACHE:
        _CACHE["nc"] = _build()
    return _CACHE["nc"]


def _prep_inputs(x, W_r, W_i):
    from ml_dtypes import bfloat16

    x = np.asarray(x, dtype=np.float32)
    Wr = np.asarray(W_r, dtype=np.float32)
    Wi = np.asarray(W_i, dtype=np.float32)
    Ws = Wr + Wi

    # xP[ms*128+p, k*256+c] = x[ms*256+c, k*128+p]
    xP = np.ascontiguousarray(
        x.reshape(N // MSUP, MSUP, KT, P).transpose(0, 3, 2, 1)
        .reshape(N // MSUP * P, KT * MSUP)
    ).astype(bfloat16)

    # wP[a*128+p, j*128+c] = W[j*128+p, a*128+c]
    def p2(W):
        return np.ascontiguousarray(
            W.reshape(KT, P, KT, P).transpose(2, 1, 0, 3).reshape(N, N)
        ).astype(bfloat16)

    # wT_c[p, k*512+n] = W[c*512+n, k*128+p]
    def p1(W, c):
        blk = W[c * SH : (c + 1) * SH, :].T  # [4096 (k), 512 (n)]
        return np.ascontiguousarray(
            blk.reshape(KT, P, SH).transpose(1, 0, 2).reshape(P, KT * SH)
        ).astype(bfloat16)

    wrP, wiP, wsP = p2(Wr), p2(Wi), p2(Ws)
    in_maps = []
    for c in range(NCORES):
        in_maps.append(
            {
                "xP": xP,
                "wrT": p1(Wr, c),
                "wiT": p1(Wi, c),
                "wrP": wrP,
                "wiP": wiP,
                "wsP": wsP,
            }
        )
    return in_maps


def kernel(x, W_r, W_i, **run_kwargs):
    from concourse.bass_utils import run_bass_kernel_spmd

    nc = _get_nc()
    in_maps = _prep_inputs(x, W_r, W_i)
    out = run_bass_kernel_spmd(nc, in_maps, list(range(NCORES)), **run_kwargs)
    res = out.results

    full = np.empty((N, 2 * N), dtype=np.float32)
    for c in range(NCORES):
        full[:, c * SH : (c + 1) * SH] = res[c]["out_r"]
        full[:, N + c * SH : N + (c + 1) * SH] = res[c]["out_i"]
    if run_kwargs:
        _CACHE["last_result"] = out
    return full


# revision 5
# speedup vs baseline: 1.1776x; 1.1776x over previous
"""Trainium2 Bass kernel for nn_LinearNet (complex double-linear).

Reference math (N = 4096):
    R_r = x @ W_r^T          R_i = x @ W_i^T
    C_r = W_r^T @ R_r - W_i^T @ R_i
    C_i = W_r^T @ R_i + W_i^T @ R_r
    out = concat([C_r, C_i], axis=1)                    # [N, 2N]

Sharding: core c owns output columns S_c = [c*512, (c+1)*512) of both C_r
and C_i.  No inter-core communication.

Pass 2 uses the 3-multiplication complex trick (Karatsuba):
    t1 = W_r^T R_r,  t2 = W_i^T R_i,  t3 = (W_r+W_i)^T (R_r+R_i)
    C_r = t1 - t2,   C_i = t3 - t1 - t2
which cuts the total matmul volume from 6 to 5 N^3-units.  All matmul
operands are bf16 (full PE speed, half the DMA/SBUF of f32); PSUM
accumulation stays fp32.  R never leaves SBUF: pass 1 evacuates PSUM
straight into the bf16 SBUF tiles pass 2 consumes, and R_s = R_r + R_i
is formed on the Pool engine during pass 1.

Host-side layout prep turns every device DMA into a contiguous slab.
Startup DMAs are fine-grained so the first matmul issues right after the
~8 us runtime boot; pass 2's first weight chunk is prefetched mid-pass-1
(issued on the Pool engine after ms=0 so it doesn't compete with the
startup-critical loads).
"""

import numpy as np

N = 4096
P = 128
NCORES = 8
SH = N // NCORES  # 512 output columns per core
KT = N // P  # 32 contraction tiles
MSUP = 256  # pass-1 output-row super tile (2 PSUM pairs live)
KH = KT // 2  # k-tiles per x half-chunk

_CACHE = {}


def _build():
    import concourse.mybir as mybir
    import concourse.tile as tile
    from concourse import bacc

    f32 = mybir.dt.float32
    bf16 = mybir.dt.bfloat16
    sub = mybir.AluOpType.subtract

    nc = bacc.Bacc()
    # xP[ms*128+p, k*256+c] = x[ms*256+c, k*128+p]
    xP = nc.declare_dram_parameter("xP", [N // MSUP * P, KT * MSUP], bf16,
                                   isOutput=False)
    # wrT[p, k*512+n] = W_r[c*512+n, k*128+p]   (per-core slice)
    wrT = nc.declare_dram_parameter("wrT", [P, KT * SH], bf16, isOutput=False)
    wiT = nc.declare_dram_parameter("wiT", [P, KT * SH], bf16, isOutput=False)
    # wrP[a*128+p, j*128+c] = W_r[j*128+p, a*128+c]   (tile-transposed full W)
    wrP = nc.declare_dram_parameter("wrP", [N, N], bf16, isOutput=False)
    wiP = nc.declare_dram_parameter("wiP", [N, N], bf16, isOutput=False)
    wsP = nc.declare_dram_parameter("wsP", [N, N], bf16, isOutput=False)
    out_r = nc.declare_dram_parameter("out_r", [N, SH], f32, isOutput=True)
    out_i = nc.declare_dram_parameter("out_i", [N, SH], f32, isOutput=True)

    with tile.TileContext(nc) as tc:
        # R lives in SBUF for the whole kernel: 3 x 32 KB/partition (bf16).
        # ws0: pass-2 a=0 weight chunk, prefetched during pass 1.
        with (
            tc.tile_pool(name="rres", bufs=1) as r_pool,
            tc.tile_pool(name="ws0", bufs=1) as ws0_pool,
        ):
            rr_sb = r_pool.tile([P, KT * SH], bf16)
            ri_sb = r_pool.tile([P, KT * SH], bf16)
            rs_sb = r_pool.tile([P, KT * SH], bf16)
            wrc0 = ws0_pool.tile([P, N], bf16)
            wic0 = ws0_pool.tile([P, N], bf16)
            wsc0 = ws0_pool.tile([P, N], bf16)

            # ---------- pass 1: R[:, S_c] = x @ W[S_c, :]^T ----------
            # psum[m, n] = sum_kk x[ms*256+m, k*128+kk] * W[n, k*128+kk]
            with (
                tc.tile_pool(name="wt", bufs=1) as wt_pool,
                tc.tile_pool(name="xs", bufs=2) as xs_pool,
                tc.tile_pool(name="ps1", bufs=4, space="PSUM") as ps1,
            ):
                wrT_sb = wt_pool.tile([P, KT * SH], bf16)
                wiT_sb = wt_pool.tile([P, KT * SH], bf16)
                # fine-grained k-sliced loads, wr/wi alternating, so the
                # first matmuls can start as soon as the k=0..3 slices land
                for k in range(0, KT, 4):
                    s = slice(k * SH, (k + 4) * SH)
                    nc.scalar.dma_start(wrT_sb[:, s], wrT[:, s])
                    nc.scalar.dma_start(wiT_sb[:, s], wiT[:, s])

                msub = MSUP // P  # 2
                for ms in range(N // MSUP):  # 16
                    xc = [
                        xs_pool.tile([P, KH * MSUP], bf16, tag="xc",
                                     name=f"xc{_h}")
                        for _h in range(2)
                    ]
                    for kh in range(2):
                        for q in range(2):
                            s = slice(q * KH * MSUP // 2,
                                      (q + 1) * KH * MSUP // 2)
                            ds = slice(kh * KH * MSUP + q * KH * MSUP // 2,
                                       kh * KH * MSUP + (q + 1) * KH * MSUP // 2)
                            nc.sync.dma_start(
                                xc[kh][:, s], xP[ms * P : (ms + 1) * P, ds]
                            )
                    acc_r = [
                        ps1.tile([P, SH], f32, tag="ps_r", name=f"accr{_s}")
                        for _s in range(msub)
                    ]
                    acc_i = [
                        ps1.tile([P, SH], f32, tag="ps_i", name=f"acci{_s}")
                        for _s in range(msub)
                    ]
                    for k in range(KT):
                        kh, kk = divmod(k, KH)
                        first, last = k == 0, k == KT - 1
                        for s in range(msub):
                            lhs = xc[kh][
                                :, kk * MSUP + s * P : kk * MSUP + (s + 1) * P
                            ]
                            nc.tensor.matmul(
                                acc_r[s][:],
                                lhs,
                                wrT_sb[:, k * SH : (k + 1) * SH],
                                start=first,
                                stop=last,
                            )
                            nc.tensor.matmul(
                                acc_i[s][:],
                                lhs,
                                wiT_sb[:, k * SH : (k + 1) * SH],
                                start=first,
                                stop=last,
                            )
                    for s in range(msub):
                        mt = ms * msub + s
                        sl = slice(mt * SH, (mt + 1) * SH)
                        nc.scalar.copy(rr_sb[:, sl], acc_r[s][:])
                        nc.vector.tensor_copy(ri_sb[:, sl], acc_i[s][:])
                        nc.gpsimd.tensor_add(
                            rs_sb[:, sl], rr_sb[:, sl], ri_sb[:, sl]
                        )
                    if ms == 0:
                        # prefetch pass-2 a=0 weights; issued on the Pool
                        # engine AFTER ms=0's rs adds (program order) so the
                        # startup-critical loads above get the HBM first
                        nc.gpsimd.dma_start(wrc0[:], wrP[0:P, :])
                        nc.gpsimd.dma_start(wic0[:], wiP[0:P, :])
                        nc.gpsimd.dma_start(wsc0[:], wsP[0:P, :])

            # ---------- pass 2: C[:, S_c] = W^T @ R (Karatsuba) ----------
            # t1[a,b] = sum_j wr[j,a] rr[j,b]; t2: wi,ri; t3: ws,rs
            with (
                tc.tile_pool(name="ws", bufs=2) as ws_pool,
                tc.tile_pool(name="ev2", bufs=3) as ev2_pool,
                tc.tile_pool(name="ps2", bufs=2, space="PSUM") as ps2,
            ):
                for a in range(N // P):  # 32
                    asl = slice(a * P, (a + 1) * P)
                    if a == 0:
                        wrc, wic, wsc = wrc0, wic0, wsc0
                    else:
                        wrc = ws_pool.tile([P, N], bf16, tag="wrc")
                        wic = ws_pool.tile([P, N], bf16, tag="wic")
                        wsc = ws_pool.tile([P, N], bf16, tag="wsc")
                        nc.sync.dma_start(wrc[:], wrP[asl, :])
                        nc.scalar.dma_start(wic[:], wiP[asl, :])
                        nc.sync.dma_start(wsc[:], wsP[asl, :])
                    t1 = ps2.tile([P, SH], f32, tag="t1")
                    t2 = ps2.tile([P, SH], f32, tag="t2")
                    t3 = ps2.tile([P, SH], f32, tag="t3")
                    for j in range(KT):
                        first, last = j == 0, j == KT - 1
                        wsl = slice(j * P, (j + 1) * P)
                        rsl = slice(j * SH, (j + 1) * SH)
                        nc.tensor.matmul(
                            t1[:], wrc[:, wsl], rr_sb[:, rsl],
                            start=first, stop=last,
                        )
                        nc.tensor.matmul(
                            t2[:], wic[:, wsl], ri_sb[:, rsl],
                            start=first, stop=last,
                        )
                        nc.tensor.matmul(
                            t3[:], wsc[:, wsl], rs_sb[:, rsl],
                            start=first, stop=last,
                        )
                    s1 = ev2_pool.tile([P, SH], f32, tag="s1")
                    cr = ev2_pool.tile([P, SH], f32, tag="cr")
                    ci = ev2_pool.tile([P, SH], f32, tag="ci")
                    nc.scalar.copy(s1[:], t1[:])
                    nc.vector.tensor_tensor(cr[:], s1[:], t2[:], sub)
                    nc.vector.tensor_tensor(ci[:], t3[:], s1[:], sub)
                    nc.vector.tensor_tensor(ci[:], ci[:], t2[:], sub)
                    # outputs go on the Pool/SWDGE path: their triggers wait
                    # on cr/ci, and on the strict-FIFO ACT ring that wait
                    # would block the next wic prefetch trigger
                    nc.gpsimd.dma_start(out_r[asl, :], cr[:])
                    nc.gpsimd.dma_start(out_i[asl, :], ci[:])

    nc.finalize()
    return nc


def _get_nc():
    if "nc" not in _CACHE:
        _CACHE["nc"] = _build()
    return _CACHE["nc"]


def _prep_inputs(x, W_r, W_i):
    from ml_dtypes import bfloat16

    x = np.asarray(x, dtype=np.float32)
    Wr = np.asarray(W_r, dtype=np.float32)
    Wi = np.asarray(W_i, dtype=np.float32)
    Ws = Wr + Wi

    # xP[ms*128+p, k*256+c] = x[ms*256+c, k*128+p]
    xP = np.ascontiguousarray(
        x.reshape(N // MSUP, MSUP, KT, P).transpose(0, 3, 2, 1)
        .reshape(N // MSUP * P, KT * MSUP)
    ).astype(bfloat16)

    # wP[a*128+p, j*128+c] = W[j*128+p, a*128+c]
    def p2(W):
        return np.ascontiguousarray(
            W.reshape(KT, P, KT, P).transpose(2, 1, 0, 3).reshape(N, N)
        ).astype(bfloat16)

    # wT_c[p, k*512+n] = W[c*512+n, k*128+p]
    def p1(W, c):
        blk = W[c * SH : (c + 1) * SH, :].T  # [4096 (k), 512 (n)]
        return np.ascontiguousarray(
            blk.reshape(KT, P, SH).transpose(1, 0, 2).reshape(P, KT * SH)
        ).astype(bfloat16)

    wrP, wiP, wsP = p2(Wr), p2(Wi), p2(Ws)
    in_maps = []
    for c in range(NCORES):
        in_maps.append(
            {
                "xP": xP,
                "wrT": p1(Wr, c),
                "wiT": p1(Wi, c),
                "wrP": wrP,
                "wiP": wiP,
                "wsP": wsP,
            }
        )
    return in_maps


def kernel(x, W_r, W_i, **run_kwargs):
    from concourse.bass_utils import run_bass_kernel_spmd

    nc = _get_nc()
    in_maps = _prep_inputs(x, W_r, W_i)
    out = run_bass_kernel_spmd(nc, in_maps, list(range(NCORES)), **run_kwargs)
    res = out.results

    full = np.empty((N, 2 * N), dtype=np.float32)
    for c in range(NCORES):
        full[:, c * SH : (c + 1) * SH] = res[c]["out_r"]
        full[:, N + c * SH : N + (c + 1) * SH] = res[c]["out_i"]
    if run_kwargs:
        _CACHE["last_result"] = out
    return full
